# revision 1
# baseline (speedup 1.0000x reference)
"""Trainium2 Bass kernel for nn_CAiA_v3 (dual-stream attention block).

Self-contained: hardcodes shapes, shards batch B=256 across 8 NeuronCores
(pure data parallel), with a tiny AllReduce for the global BatchNorm
statistics. All activations flow on-device in transposed (feature x row)
layout so every GEMM contracts over partitions without transposes.
"""

from contextlib import ExitStack

import numpy as np
import ml_dtypes

import concourse.bass as bass
import concourse.bacc as bacc
import concourse.tile as tile
from concourse import mybir
from concourse.bass_utils import run_bass_kernel_spmd

BF16 = mybir.dt.bfloat16
F32 = mybir.dt.float32
AF = mybir.ActivationFunctionType
OP = mybir.AluOpType

B, HN, N1, D = 256, 12, 12, 1024
NCORES = 8
BL = B // NCORES          # 32 local batches
BH = BL * HN              # 384 (b,h) groups per core
R = BH * N1               # 4608 rows per stream per core
R2 = 2 * R                # 9216 rows (value / key path)
CH = 384                  # row chunk for embed/norm phases (32 bh, 12-aligned)
NCH = R // CH             # 12
VCH = 384                 # value-row chunk (16 bh * 24)
NVCH = R2 // VCH          # 24
QG = 4                    # bh per attention group
NQ = BH // QG             # 96 attention groups
SGQ = 8                   # quads per 32-bh supergroup
EPS = 1e-5
N_TOT = float(B * HN * D)  # BN stat count per channel (global)
SCALE = 1.0 / 32.0         # attention softmax scale = D**-0.5

_CACHE = {}


def _build(sim_mode=False):
    nc = bacc.Bacc("TRN2", target_bir_lowering=False, debug=False,
                   num_devices=NCORES)

    def din(name, shape, dt=BF16):
        return nc.declare_dram_parameter(name, list(shape), dt, isOutput=False)

    aT = din("aT", (D, R))
    bT = din("bT", (D, R))
    catT = din("catT", (D, R2))   # (bh, 24)-interleaved concat of a/b rows
    posT = din("posT", (D, R))
    ewT = din("ewT", (D, D))
    qwT = din("qwT", (D, D))
    kwT = din("kwT", (D, D))
    vwT = din("vwT", (D, D))
    owT = din("owT", (D, D))
    eb = din("eb", (D,), F32)
    qb = din("qb", (D,), F32)
    kb = din("kb", (D,), F32)
    vb = din("vb", (D,), F32)
    ob = din("ob", (D,), F32)
    bnw = din("bnw", (N1,), F32)
    bnb = din("bnb", (N1,), F32)
    lnw = din("lnw", (D,), F32)
    lnb = din("lnb", (D,), F32)

    out_r = nc.declare_dram_parameter("out_r", [R, D], F32, isOutput=True)
    out_t = nc.declare_dram_parameter("out_t", [R, D], F32, isOutput=True)

    # internal DRAM
    XT = [nc.dram_tensor(f"XT{t}", [D, R], BF16) for t in range(2)]
    qT2 = [nc.dram_tensor(f"qT2{t}", [D, R], BF16) for t in range(2)]
    kT2 = [nc.dram_tensor(f"kT2{t}", [D, R], BF16) for t in range(2)]
    val = nc.dram_tensor("val", [R2, D], BF16)
    attT = [nc.dram_tensor(f"attT{t}", [D, R], BF16) for t in range(2)]
    cc_in = nc.dram_tensor("cc_in", [2, 24], F32)
    cc_out = nc.dram_tensor("cc_out", [2, 24], F32, addr_space="Shared")

    v3 = lambda h: h[:].rearrange("(dt p) c -> p dt c", p=128)
    aTv, bTv, posTv = v3(aT), v3(bT), v3(posT)
    XTv = [v3(x) for x in XT]
    qT2v = [v3(x) for x in qT2]
    kT2v = [v3(x) for x in kT2]
    attTv = [v3(x) for x in attT]
    inTv = [aTv, bTv]
    catTv = v3(catT)

    with tile.TileContext(nc) as tc, ExitStack() as ctx:
        # ---------- constants / weights resident in SBUF ----------
        const = ctx.enter_context(tc.tile_pool(name="const", bufs=1))
        w_sb = {}
        _w_pending = []
        for name, h in (("qw", qwT), ("kw", kwT), ("vw", vwT),
                        ("ow", owT)):
            t_ = const.tile([128, 8, D], BF16, tag=f"w_{name}",
                            name=f"w_{name}")
            _w_pending.append((t_, h))
            w_sb[name] = t_

        def colvec(h, tag):  # (D,) -> [128, 8] per-partition columns
            t_ = const.tile([128, 8], F32, tag=tag, name=tag)
            nc.sync.dma_start(out=t_[:],
                              in_=h[:].rearrange("(t p) -> p t", p=128))
            return t_

        eb_sb = colvec(eb, "eb_sb")
        qb_sb = colvec(qb, "qb_sb")
        kb_sb = colvec(kb, "kb_sb")
        lnw_sb = colvec(lnw, "lnw_sb")
        lnb_sb = colvec(lnb, "lnb_sb")

        def bcast128(h, n, tag, dt=F32):  # (n,) -> [128, n] replicated
            t_ = const.tile([128, n], dt, tag=tag, name=tag)
            src = bass.AP(tensor=h[:].tensor, offset=h[:].offset,
                          ap=[[0, 128], [1, n]])
            eng = nc.sync if dt == F32 else nc.gpsimd
            eng.dma_start(out=t_[:], in_=src)
            return t_

        vb_sb = bcast128(vb, D, "vb_sb", BF16)
        ob_sb = bcast128(ob, D, "ob_sb", BF16)
        bnw_sb = bcast128(bnw, N1, "bnw_sb")
        bnb_sb = bcast128(bnb, N1, "bnb_sb")

        ones_b = const.tile([128, 128], BF16, tag="ones_b", name="ones_b")
        nc.vector.memset(ones_b[:], 1.0)
        ones_f = const.tile([128, 128], F32, tag="ones_f", name="ones_f")
        nc.vector.memset(ones_f[:], 1.0)
        eps128 = const.tile([128, 1], F32, tag="eps128", name="eps128")
        nc.vector.memset(eps128[:], EPS)

        # BN stat accumulators: per (type, jt): [128, 24] = sum | sumsq
        acc = [[const.tile([128, 24], F32, tag=f"acc{t}_{j}",
                           name=f"acc{t}_{j}") for j in range(8)]
               for t in range(2)]
        for t in range(2):
            for j in range(8):
                nc.vector.memset(acc[t][j][:], 0.0)

        # alpha/beta live through P2 -> const pool (bf16 for DVE 2x mode)
        alpha128 = [const.tile([128, N1], BF16, tag=f"al{t}", name=f"al{t}")
                    for t in range(2)]
        beta128 = [const.tile([128, N1], BF16, tag=f"be{t}", name=f"be{t}")
                   for t in range(2)]

        # ---------- P1: embed GEMM (X.T = ewT.T @ a.T) + BN partial stats ----
        with tc.tile_pool(name="p1in", bufs=2) as p1in, \
             tc.tile_pool(name="p1wk", bufs=3) as p1wk, \
             tc.tile_pool(name="ps1", bufs=3, space="PSUM") as ps1:
            ew_sb = p1in.tile([128, 8, D], BF16, tag="w_ew", name="w_ew",
                              bufs=1)
            nc.sync.dma_start(out=ew_sb[:], in_=v3(ewT))
            for t in range(2):
                for c in range(NCH):
                    ain = p1in.tile([128, 8, CH], BF16, tag="ain", name="ain")
                    nc.sync.dma_start(
                        out=ain[:], in_=inTv[t][:, :, c * CH:(c + 1) * CH])
                    xev = p1wk.tile([128, 8, CH], BF16, tag="xev", name="xev")
                    for jt in range(8):
                        ps = ps1.tile([128, CH], F32, tag="ps", name="ps")
                        for d in range(8):
                            nc.tensor.matmul(
                                ps[:],
                                ew_sb[:, d, jt * 128:(jt + 1) * 128],
                                ain[:, d, :], start=(d == 0), stop=(d == 7))
                        xsb = xev[:, jt, :]
                        nc.scalar.activation(xsb, ps[:], AF.Identity,
                                             bias=eb_sb[:, jt:jt + 1],
                                             scale=1.0)
                        sq = p1wk.tile([128, CH], BF16, tag="sq", name="sq")
                        nc.scalar.square(sq[:], xsb)
                        rs = p1wk.tile([128, N1], F32, tag="rs", name="rs")
                        rq = p1wk.tile([128, N1], F32, tag="rq", name="rq")
                        nc.vector.tensor_reduce(
                            rs[:], xsb.rearrange("p (bh n) -> p n bh", n=N1),
                            axis=mybir.AxisListType.X, op=OP.add)
                        nc.vector.tensor_reduce(
                            rq[:], sq[:].rearrange("p (bh n) -> p n bh", n=N1),
                            axis=mybir.AxisListType.X, op=OP.add)
                        nc.vector.tensor_add(acc[t][jt][:, 0:N1],
                                             acc[t][jt][:, 0:N1], rs[:])
                        nc.vector.tensor_add(acc[t][jt][:, N1:24],
                                             acc[t][jt][:, N1:24], rq[:])
                    nc.sync.dma_start(
                        out=XTv[t][:, :, c * CH:(c + 1) * CH], in_=xev[:])

        for t_, h in _w_pending:
            nc.sync.dma_start(out=t_[:], in_=v3(h))

        # ---------- BN stats: reduce, AllReduce, alpha/beta ----------
        with tc.tile_pool(name="stt", bufs=1) as stt, \
             tc.tile_pool(name="ps_st", bufs=1, space="PSUM") as ps_st:
            for t in range(2):
                s_all = stt.tile([128, 24], F32, tag=f"sall{t}",
                                 name=f"sall{t}")
                nc.vector.tensor_copy(s_all[:], acc[t][0][:])
                for j in range(1, 8):
                    nc.vector.tensor_add(s_all[:], s_all[:], acc[t][j][:])
                red = ps_st.tile([128, 24], F32, tag=f"red{t}",
                                 name=f"red{t}")
                nc.tensor.matmul(red[:], ones_f[:], s_all[:],
                                 start=True, stop=True)
                redsb = stt.tile([1, 24], F32, tag=f"redsb{t}",
                                 name=f"redsb{t}")
                nc.vector.tensor_copy(redsb[:], red[0:1, :])
                nc.sync.dma_start(out=cc_in[t:t + 1, :], in_=redsb[:])
            if sim_mode:
                nc.sync.dma_start(out=cc_out[:], in_=cc_in[:])
            else:
                nc.gpsimd.collective_compute(
                    "AllReduce", OP.add, replica_groups=[list(range(NCORES))],
                    ins=[cc_in[:]], outs=[cc_out[:]])
            gst = stt.tile([128, 48], F32, tag="gst", name="gst")
            nc.sync.dma_start(
                out=gst[:],
                in_=bass.AP(tensor=cc_out[:].tensor, offset=cc_out[:].offset,
                            ap=[[0, 128], [1, 48]]))
            for t in range(2):
                S = gst[:, t * 24:t * 24 + 12]
                S2 = gst[:, t * 24 + 12:t * 24 + 24]
                mean = stt.tile([128, N1], F32, tag=f"mean{t}",
                                name=f"mean{t}")
                nc.scalar.mul(mean[:], S, 1.0 / N_TOT)
                e2 = stt.tile([128, N1], F32, tag=f"e2{t}", name=f"e2{t}")
                nc.scalar.mul(e2[:], S2, 1.0 / N_TOT)
                m2 = stt.tile([128, N1], F32, tag=f"m2{t}", name=f"m2{t}")
                nc.vector.tensor_mul(m2[:], mean[:], mean[:])
                nc.vector.tensor_sub(e2[:], e2[:], m2[:])
                sd = stt.tile([128, N1], F32, tag=f"sd{t}", name=f"sd{t}")
                nc.scalar.activation(sd[:], e2[:], AF.Sqrt, bias=eps128[:],
                                     scale=1.0)
                nc.vector.reciprocal(sd[:], sd[:])
                nc.vector.tensor_mul(alpha128[t][:], sd[:], bnw_sb[:])
                nc.vector.tensor_mul(beta128[t][:], alpha128[t][:], mean[:])
                nc.vector.tensor_sub(beta128[t][:], bnb_sb[:], beta128[t][:])

        # ---------- fused main loop: per 32-bh supergroup ----------
        # P3 (LN + value GEMM, 2x 16-bh chunks) -> P2 (BN apply + q/k GEMMs,
        # evacuated straight into SBUF stack tiles) -> P4 (attention) ->
        # P5 (output projection). Only `val` round-trips DRAM (needed for
        # the 32-stride partition shuffle); q/k/att stay in SBUF.
        with tc.tile_pool(name="fin", bufs=2) as fin, \
             tc.tile_pool(name="fwk", bufs=2) as fwk, \
             tc.tile_pool(name="fst", bufs=1) as fst, \
             tc.tile_pool(name="fas", bufs=2) as fas, \
             tc.tile_pool(name="fps", bufs=4, space="PSUM") as fps, \
             tc.tile_pool(name="fpa", bufs=4, space="PSUM") as fpa:
            outs = [out_r, out_t]
            def p3_stage_a(vc):
                stt_ = fin.tile([128, 8, VCH], BF16, tag="stt_", name="stt_")
                nc.sync.dma_start(
                    out=stt_[:], in_=catTv[:, :, vc * VCH:(vc + 1) * VCH])
                sqt = fwk.tile([128, 8, VCH], BF16, tag="sqt", name="sqt")
                for d in range(8):
                    nc.scalar.square(sqt[:, d, :], stt_[:, d, :])
                ssum = fps.tile([128, VCH], F32, tag="ps", name="ssum")
                for d in range(8):
                    nc.tensor.matmul(ssum[:], ones_b[:], stt_[:, d, :],
                                     start=(d == 0), stop=(d == 7))
                s2sum = fps.tile([128, VCH], F32, tag="ps", name="s2sum")
                for d in range(8):
                    nc.tensor.matmul(s2sum[:], ones_b[:], sqt[:, d, :],
                                     start=(d == 0), stop=(d == 7))
                mrow = fst.tile([128, VCH], F32, tag="mrow", name="mrow")
                nc.scalar.mul(mrow[:], ssum[:], 1.0 / D)
                crow = fst.tile([128, VCH], F32, tag="crow", name="crow")
                nc.scalar.mul(crow[:], s2sum[:], 1.0 / D)
                m2r = fst.tile([128, VCH], F32, tag="m2r", name="m2r")
                nc.vector.tensor_mul(m2r[:], mrow[:], mrow[:])
                nc.vector.tensor_sub(crow[:], crow[:], m2r[:])
                nc.scalar.activation(crow[:], crow[:], AF.Sqrt,
                                     bias=eps128[:], scale=1.0)
                nc.vector.reciprocal(crow[:], crow[:])
                drow = fst.tile([128, VCH], F32, tag="drow", name="drow")
                nc.vector.tensor_mul(drow[:], mrow[:], crow[:])
                crow_b = fwk.tile([128, VCH], BF16, tag="crow_b",
                                  name="crow_b")
                nc.scalar.copy(crow_b[:], crow[:])
                drow_b = fwk.tile([128, VCH], BF16, tag="drow_b",
                                  name="drow_b")
                nc.scalar.mul(drow_b[:], drow[:], -1.0)
                return dict(stt_=stt_, sqt=sqt, crow_b=crow_b, drow_b=drow_b)

            def p3_stage_b(vc, sA):
                stt_, sqt = sA["stt_"], sA["sqt"]
                crow_b, drow_b = sA["crow_b"], sA["drow_b"]
                for d in range(8):
                    nc.vector.tensor_mul(sqt[:, d, :], stt_[:, d, :],
                                         crow_b[:])
                    nc.vector.tensor_add(sqt[:, d, :], sqt[:, d, :],
                                         drow_b[:])
                    nc.scalar.activation(sqt[:, d, :], sqt[:, d, :],
                                         AF.Identity,
                                         bias=lnb_sb[:, d:d + 1],
                                         scale=lnw_sb[:, d:d + 1])
                for mt in range(3):
                    for n2 in range(2):
                        pv = fps.tile([128, 512], F32, tag="ps", name="pv")
                        for d in range(8):
                            nc.tensor.matmul(
                                pv[:], sqt[:, d, mt * 128:(mt + 1) * 128],
                                w_sb["vw"][:, d, n2 * 512:(n2 + 1) * 512],
                                start=(d == 0), stop=(d == 7))
                        ev = fwk.tile([128, 512], BF16, tag="vev",
                                      name="vev")
                        nc.vector.tensor_add(
                            ev[:], pv[:], vb_sb[:, n2 * 512:(n2 + 1) * 512])
                        nc.sync.dma_start(
                            out=val[vc * VCH + mt * 128:
                                    vc * VCH + (mt + 1) * 128,
                                    n2 * 512:(n2 + 1) * 512],
                            in_=ev[:])

            prevA = p3_stage_a(0)
            for vc in range(1, NVCH):
                curA = p3_stage_a(vc)
                p3_stage_b(vc - 1, prevA)
                prevA = curA
            p3_stage_b(NVCH - 1, prevA)
            for c in range(NCH):
                # ----- P2: BN apply + pos, q/k GEMMs into SBUF stacks -----
                qstk = fst.tile([128, 8, BL, 24], BF16, tag="qstk",
                                name="qstk")
                kstk = fst.tile([128, 8, BL, 32], BF16, tag="kstk",
                                name="kstk")
                p_ = fin.tile([128, 8, CH], BF16, tag="p_", name="p_")
                nc.sync.dma_start(
                    out=p_[:], in_=posTv[:, :, c * CH:(c + 1) * CH])
                for t in range(2):
                    x_ = fin.tile([128, 8, CH], BF16, tag="x_", name="x_")
                    nc.sync.dma_start(
                        out=x_[:], in_=XTv[t][:, :, c * CH:(c + 1) * CH])
                    nrm = fwk.tile([128, 8, CH], BF16, tag="nrm", name="nrm")
                    ab = alpha128[t][:, None, :].to_broadcast(
                        (128, CH // N1, N1))
                    bb = beta128[t][:, None, :].to_broadcast(
                        (128, CH // N1, N1))
                    for d in range(8):
                        xv = x_[:, d, :].rearrange("p (bh n) -> p bh n", n=N1)
                        nv = nrm[:, d, :].rearrange("p (bh n) -> p bh n", n=N1)
                        nc.vector.tensor_mul(nv, xv, ab)
                        nc.vector.tensor_add(nv, nv, bb)
                        nc.vector.tensor_add(nrm[:, d, :], nrm[:, d, :],
                                             p_[:, d, :])
                    for w, bias_sb, stk in (("qw", qb_sb, qstk),
                                            ("kw", kb_sb, kstk)):
                        for jt in range(8):
                            pq = fps.tile([128, CH], F32, tag="ps", name="pq")
                            for d in range(8):
                                nc.tensor.matmul(
                                    pq[:],
                                    w_sb[w][:, d, jt * 128:(jt + 1) * 128],
                                    nrm[:, d, :], start=(d == 0), stop=(d == 7))
                            nc.scalar.activation(
                                stk[:, jt, :, 12 * t:12 * t + 12],
                                pq[:].rearrange("p (bh n) -> p bh n", n=N1),
                                AF.Identity, bias=bias_sb[:, jt:jt + 1],
                                scale=1.0)

                qflat = qstk[:].rearrange("p dt bh j -> p dt (bh j)")
                kflat = kstk[:].rearrange("p dt bh j -> p dt (bh j)")
                # ----- P4: attention over 8 quads (A/B pipelined) -----
                att_sup = fas.tile([128, 8, 2, CH], BF16, tag="att_sup",
                                   name="att_sup")

                def p4_stage_a(gq):
                    g = c * SGQ + gq
                    sv = fwk.tile([128, D], BF16, tag="sv", name="sv")
                    nc.gpsimd.memset(sv[:], 0.0)
                    for b in range(QG):
                        nc.sync.dma_start(
                            out=sv[32 * b:32 * b + 24, :],
                            in_=val[g * 96 + 24 * b: g * 96 + 24 * (b + 1), :])
                    eT = fwk.tile([128, 96], BF16, tag="eT", name="eT")
                    nc.gpsimd.memset(eT[:], 0.0)
                    pl = fpa.tile([128, 96], F32, tag="pa", name="pl")
                    for d in range(8):
                        nc.tensor.matmul(
                            pl[:],
                            kflat[:, d, gq * 128:(gq + 1) * 128],
                            qflat[:, d, gq * 96:(gq + 1) * 96],
                            start=(d == 0), stop=(d == 7))
                    for b in range(QG):
                        nc.scalar.activation(
                            eT[32 * b:32 * b + 24, 24 * b:24 * (b + 1)],
                            pl[32 * b:32 * b + 24, 24 * b:24 * (b + 1)],
                            AF.Exp)
                    return dict(sv=sv, eT=eT)

                def p4_stage_b(gq, sA):
                    sv, eT = sA["sv"], sA["eT"]
                    pss = fpa.tile([128, 96], F32, tag="pa", name="pss")
                    nc.tensor.matmul(pss[:], ones_b[:], eT[:],
                                     start=True, stop=True)
                    rB = fwk.tile([128, 96], F32, tag="rB", name="rB")
                    nc.vector.reciprocal(rB[:], pss[:])
                    rBv = rB[:].rearrange("p (b t n) -> p b t n", t=2, n=12)
                    for vt in range(8):
                        pa = fpa.tile([128, 96], F32, tag="pa", name="pa")
                        nc.tensor.matmul(pa[:], sv[:, vt * 128:(vt + 1) * 128],
                                         eT[:], start=True, stop=True)
                        nc.vector.tensor_mul(
                            att_sup[:, vt, :,
                                    gq * 48:gq * 48 + 48].rearrange(
                                "p t (b n) -> p b t n", n=12),
                            pa[:].rearrange("p (b t n) -> p b t n", t=2, n=12),
                            rBv)

                prevQ = p4_stage_a(0)
                for gq in range(1, SGQ):
                    curQ = p4_stage_a(gq)
                    p4_stage_b(gq - 1, prevQ)
                    prevQ = curQ
                p4_stage_b(SGQ - 1, prevQ)

                # ----- P5: output projection for this supergroup -----
                for t in range(2):
                    for mi in range(3):
                        for n2 in range(2):
                            po = fps.tile([128, 512], F32, tag="ps", name="po")
                            for vt in range(8):
                                nc.tensor.matmul(
                                    po[:],
                                    att_sup[:, vt, t,
                                            mi * 128:(mi + 1) * 128],
                                    w_sb["ow"][:, vt, n2 * 512:(n2 + 1) * 512],
                                    start=(vt == 0), stop=(vt == 7))
                            oe = fwk.tile([128, 512], F32, tag="oe", name="oe")
                            nc.vector.tensor_add(
                                oe[:], po[:],
                                ob_sb[:, n2 * 512:(n2 + 1) * 512])
                            nc.sync.dma_start(
                                out=outs[t][:][c * CH + mi * 128:
                                               c * CH + (mi + 1) * 128,
                                               n2 * 512:(n2 + 1) * 512],
                                in_=oe[:])

    nc.compile()
    return nc


def _get_nc():
    if "nc" not in _CACHE:
        _CACHE["nc"] = _build()
    return _CACHE["nc"]


def _prep_in_maps(attn_rgb, attn_tir, pos_emb, embed_w, embed_b, bn_w, bn_b,
                  ln_w, ln_b, v_w, v_b, q_w, q_b, k_w, k_b, out_w, out_b):
    bf16 = ml_dtypes.bfloat16
    f32 = np.float32

    def tb(x):  # (rows, D) f32 -> (D, rows) bf16 contiguous
        return np.ascontiguousarray(np.asarray(x, f32).astype(bf16).T)

    ar = np.asarray(attn_rgb, f32).reshape(B * HN * N1, D)
    at = np.asarray(attn_tir, f32).reshape(B * HN * N1, D)
    arT = tb(ar)
    atT = tb(at)
    # (bh, 24)-interleaved concat for the LN/value and attention paths
    cat = np.empty((B * HN, 24, D), f32)
    cat[:, 0:12] = ar.reshape(B * HN, N1, D)
    cat[:, 12:24] = at.reshape(B * HN, N1, D)
    catT = tb(cat.reshape(-1, D))
    pe = np.asarray(pos_emb, f32)[0]                      # (B, N1, D)
    posr = np.broadcast_to(pe[:, None, :, :], (B, HN, N1, D)).reshape(-1, D)
    posT = tb(posr)

    wT = lambda w: np.ascontiguousarray(np.asarray(w, f32).T.astype(bf16))
    shared = {
        "ewT": wT(embed_w),
        "qwT": np.ascontiguousarray(
            (np.asarray(q_w, f32).T * np.float32(SCALE)).astype(bf16)),
        "kwT": wT(k_w),
        "vwT": wT(v_w),
        "owT": wT(out_w),
        "eb": np.asarray(embed_b, f32),
        "qb": np.asarray(q_b, f32) * np.float32(SCALE),
        "kb": np.asarray(k_b, f32),
        "vb": np.asarray(v_b, f32),
        "ob": np.asarray(out_b, f32),
        "bnw": np.asarray(bn_w, f32),
        "bnb": np.asarray(bn_b, f32),
        "lnw": np.asarray(ln_w, f32),
        "lnb": np.asarray(ln_b, f32),
    }
    in_maps = []
    for c in range(NCORES):
        sl = slice(c * R, (c + 1) * R)
        sl2 = slice(c * R2, (c + 1) * R2)
        in_maps.append({
            "aT": np.ascontiguousarray(arT[:, sl]),
            "bT": np.ascontiguousarray(atT[:, sl]),
            "catT": np.ascontiguousarray(catT[:, sl2]),
            "posT": np.ascontiguousarray(posT[:, sl]),
            **shared,
        })
    return in_maps


def kernel(**inputs):
    in_maps = _prep_in_maps(**inputs)
    nc = _get_nc()
    res = run_bass_kernel_spmd(nc, in_maps, list(range(NCORES)))
    o_r = np.concatenate([res.results[c]["out_r"] for c in range(NCORES)],
                         axis=0).reshape(B, HN, N1, D)
    o_t = np.concatenate([res.results[c]["out_t"] for c in range(NCORES)],
                         axis=0).reshape(B, HN, N1, D)
    return o_r, o_t

